# revision 7
# baseline (speedup 1.0000x reference)
"""Trainium2 Bass kernel for 12-head cross-attention with additive bias.

Reference computation (b=2, n=m=2048, e=768, h=12, d=64), all fp32:
    q  = x @ Wq.T;  kv = context @ Wkv.T;  k, v = split(kv)
    sim  = (q_h @ k_h.T) * d**-0.5 + attn_bias
    out_h = softmax(sim) @ v_h
    y = concat_heads(out) @ Wout.T + b_out
Sharding: 8 cores = 2 batches x 4 head-groups (3 heads each).  Each core
computes the projections for its head group, attention for its 3 heads, and
a partial output projection y_part[b] = out_g @ Wout[:, cols_g].T.  The host
sums the 4 per-group partials of each batch and adds b_out.

Key device-side structure (all bf16 on-chip, softmax scale folded into Wq):
    QT[d,q] / KT[d,m] = WT-chunks.T @ xT-chunks          (PE)
    ST[m,q] = KT-slice.T @ QT  -- d=64 contraction, so every S matmul is a
        64-row PE tile.  Heads 0/1 are packed in the two partition halves of
        qt[0]/kt[0]; their S matmuls for the same m-tile are issued
        back-to-back as tile (0,0) / (64,0), which the PE runs CONCURRENTLY
        (2x row tiling).  Head 2 is replicated into both partition halves so
        two m-tiles of it pair the same way.
    PT = exp(ST)                                          (ScalarE, PSUM->SBUF)
    PT *= eb tile                                         (DVE, bf16 2x)
        exp(attn_bias) ships as uint8 with a per-(h, n) scale -- softmax
        normalization cancels any per-query factor exactly -- and the SWDGE
        DMA casts u8 -> bf16 on the way into SBUF (halves HBM reads).
    OT[d,q] += V_aug-slice.T @ PT   (V_aug has a ones column -> denominators)
    y[q,j] += OT-chunks.T @ WoutT-chunks   (normalized by 1/denom first)
"""

import numpy as np
import ml_dtypes

import concourse.bacc as bacc
import concourse.mybir as mybir
import concourse.tile as tile
from concourse.bass_utils import run_bass_kernel_spmd

BF16 = ml_dtypes.bfloat16

B, N, M, E = 2, 2048, 2048, 768
HEADS = 12
D = 64                      # head dim
GROUPS = 4                  # head groups (cores per batch)
HG = HEADS // GROUPS        # heads per group = 3
CG = HG * D                 # channels per group = 192
NCORES = 8

P = 128                     # partitions
QC = 512                    # q free-dim chunk
NQ = N // QC                # 4 q-chunks
MT = M // P                 # 16 m-tiles
EC = E // P                 # 6 contraction chunks
JC = 384                    # output-proj free chunk
NJ = E // JC                # 2 output-proj chunks

_CACHED_NC = None


def build_nc(reps=1, sp_bufs=2, gen_bufs=2, pt_bufs=8, eb_bufs=3,
             ydt_bf16=1, startup=2, eb_chunks=2, odelay=3, pieces=2,
             tail_split=1, act_warm=1, pvburst=1):
    import collections as _collections
    f32 = mybir.dt.float32
    bf16 = mybir.dt.bfloat16
    u8 = mybir.dt.uint8
    y_dt = bf16 if ydt_bf16 else f32

    nc = bacc.Bacc("TRN2", debug=False)
    xT = nc.dram_tensor("xT", [E, N], bf16, kind="ExternalInput")
    cT = nc.dram_tensor("cT", [E, M], bf16, kind="ExternalInput")
    # exp(bias) quantized to u8 with a per-(h, n) scale (softmax cancels any
    # per-query-column factor).  Slab 0: heads 0/1 interleaved per m-tile
    # (cols = mt*2QC + h*QC + qc) for m-tiles 0-7; slab 1: same, m-tiles
    # 8-15; slab 2: head 2 (cols = mt*QC + qc).
    expbT = nc.dram_tensor("expbT", [HG, NQ, P, MT * QC], u8,
                           kind="ExternalInput")
    # w{q,k,v}T pre-tiled on host to [P, EC*CG]: one fat descriptor per
    # partition line instead of EC*CG/P thin ones
    wqT = nc.dram_tensor("wqT", [P, EC * CG], bf16, kind="ExternalInput")
    wkT = nc.dram_tensor("wkT", [P, EC * CG], bf16, kind="ExternalInput")
    wvT = nc.dram_tensor("wvT", [P, EC * CG], bf16, kind="ExternalInput")
    woT = nc.dram_tensor("woT", [CG, E], bf16, kind="ExternalInput")
    y = nc.dram_tensor("y", [N, E], y_dt, kind="ExternalOutput")

    with tile.TileContext(nc) as tc:
        with (
            tc.tile_pool(name="big", bufs=1) as big,
            tc.tile_pool(name="eb_pool", bufs=eb_bufs) as eb_pool,
            tc.tile_pool(name="pt_pool", bufs=pt_bufs) as pt_pool,
            tc.tile_pool(name="small", bufs=4) as small,
            tc.tile_pool(name="ysb_pool", bufs=4) as ysb_pool,
            tc.tile_pool(name="ps_sp", bufs=sp_bufs, space="PSUM") as ps_sp,
            tc.tile_pool(name="ps_gen", bufs=gen_bufs, space="PSUM") as ps_gen,
            tc.tile_pool(name="ps_o", bufs=2, space="PSUM") as ps_o,
        ):
          for _rep in range(reps):
            # ---- load inputs (context first: KT/V unblock the S matmuls) ----
            wq_sb = big.tile([P, EC, CG], bf16)
            wk_sb = big.tile([P, EC, CG], bf16)
            wv_sb = big.tile([P, EC, CG], bf16)
            wo_sb = big.tile([P, 2, E], bf16)
            c_sb = [big.tile([P, M], bf16, name=f"c{_e}") for _e in range(EC)]
            x_sb = [big.tile([P, N], bf16, name=f"x{_e}") for _e in range(EC)]

            def dma_c(q0, q1):
                for e in range(EC):
                    nc.sync.dma_start(out=c_sb[e][:, q0:q1],
                                      in_=cT[e * P : (e + 1) * P, q0:q1])

            def dma_x(q0, q1):
                for e in range(EC):
                    nc.sync.dma_start(out=x_sb[e][:, q0:q1],
                                      in_=xT[e * P : (e + 1) * P, q0:q1])

            qt = [big.tile([P, N], bf16, name=f"qt{_h}") for _h in (0, 2)]
            qt = {0: qt[0], 2: qt[1]}
            ot01 = big.tile([P, N], bf16)
            ot2 = big.tile([D, N], bf16)

            kt = [big.tile([P, M], bf16, name=f"kt{_h}") for _h in (0, 2)]
            kt = {0: kt[0], 2: kt[1]}

            def gen_proj(jq_, w_sb, dst, src):
                # Q or K projection for one 512-chunk, as resumable pieces.
                # Heads 0/1 stay packed in the two partition halves of
                # dst[0]; head 2 lands in dst[2][0:D] and is replicated to
                # dst[2][D:P] so its S matmuls can pair as PE row tiles.
                qs_ = slice(jq_ * QC, (jq_ + 1) * QC)
                pr01 = ps_gen.tile([P, QC], f32, tag="gen", name="pr01")
                for e in range(EC):
                    nc.tensor.matmul(
                        pr01[:], w_sb[:, e, 0:P], src[e][:, qs_],
                        start=(e == 0), stop=(e == EC - 1),
                    )
                    yield
                nc.vector.tensor_copy(dst[0][:, qs_], pr01[:])
                yield
                pr2 = ps_gen.tile([D, QC], f32, tag="gen", name="pr2")
                for e in range(EC):
                    nc.tensor.matmul(
                        pr2[:], w_sb[:, e, P:CG], src[e][:, qs_],
                        start=(e == 0), stop=(e == EC - 1),
                    )
                    yield
                nc.vector.tensor_copy(dst[2][0:D, qs_], pr2[:])
                yield
                nc.vector.tensor_copy(dst[2][D:P, qs_], dst[2][0:D, qs_])
                yield

            def gen_qtproj(jq_):
                return gen_proj(jq_, wq_sb, qt, x_sb)

            def gen_ktproj(jq_):
                return gen_proj(jq_, wk_sb, kt, c_sb)

            def run_gen(g):
                for _ in g:
                    pass

            def emit_qtproj(jq_):
                run_gen(gen_qtproj(jq_))

            def emit_ktproj(jq_):
                run_gen(gen_ktproj(jq_))

            fill_q = _collections.deque()

            def drain(n):
                for _ in range(n):
                    while fill_q:
                        try:
                            next(fill_q[0])
                            break
                        except StopIteration:
                            fill_q.popleft()
                    if not fill_q:
                        return

            def drain_all():
                while fill_q:
                    run_gen(fill_q.popleft())

            def eb_fetch(h_, jq_):
                # u8 -> bf16 cast during DMA: SWDGE (gpsimd) only
                eb_sb = eb_pool.tile([P, MT * QC], bf16, tag="eb")
                step = MT * QC // eb_chunks
                for ci in range(eb_chunks):
                    nc.gpsimd.dma_start(
                        out=eb_sb[:, ci * step : (ci + 1) * step],
                        in_=expbT[h_, jq_, :, ci * step : (ci + 1) * step],
                    )
                return eb_sb

            # ---- V projection helper (emitted per m-tile inside jq0) ----
            v_sb = big.tile([P, MT, HG, D + 1], bf16)

            def emit_vproj(mt):
                pv = ps_gen.tile([P, CG], f32, tag="gen", name="pv")
                for e in range(EC):
                    nc.tensor.matmul(
                        pv[:], c_sb[e][:, mt * P : (mt + 1) * P], wv_sb[:, e, :],
                        start=(e == 0), stop=(e == EC - 1),
                    )
                nc.vector.tensor_copy(
                    v_sb[:, mt, :, 0:D], pv.rearrange("p (h d) -> p h d", d=D)
                )

            def gen_ygroup(qtile, act_copy=False):
                qsl = slice(qtile * P, (qtile + 1) * P)
                y_sb = ysb_pool.tile([P, E], y_dt, tag="ysb", name="y_sb")
                for jn in range(NJ):
                    jsl = slice(jn * JC, (jn + 1) * JC)
                    y_ps = ps_gen.tile([P, JC], f32, tag="gen", name="y_ps")
                    nc.tensor.matmul(
                        y_ps[:], ot01[:, qsl], wo_sb[:, 0, jsl],
                        start=True, stop=False,
                    )
                    yield
                    nc.tensor.matmul(
                        y_ps[:], ot2[:, qsl], wo_sb[0 : CG - P, 1, jsl],
                        start=False, stop=True,
                    )
                    yield
                    if act_copy:
                        nc.scalar.copy(y_sb[:, jsl], y_ps[:])
                    else:
                        nc.vector.tensor_copy(y_sb[:, jsl], y_ps[:])
                    yield
                nc.sync.dma_start(out=y[qsl, :], in_=y_sb[:])

            def emit_ygroup(qtile, act_copy=False):
                run_gen(gen_ygroup(qtile, act_copy))

            eb_first = None
            eb_second = None
            if startup == 2:
                # startup-critical loads, spread across three sequencers so
                # DMA issue (~0.6us each) pipelines: K path on SP, Q path on
                # ACT, V on DVE-side SWDGE
                nc.sync.dma_start(out=wk_sb[:], in_=wkT.rearrange("p (c d) -> p c d", d=CG))
                nc.scalar.dma_start(out=wq_sb[:], in_=wqT.rearrange("p (c d) -> p c d", d=CG))
                nc.gpsimd.dma_start(out=wv_sb[:], in_=wvT.rearrange("p (c d) -> p c d", d=CG))
                for e in range(EC):
                    nc.sync.dma_start(out=c_sb[e][:, 0:QC],
                                      in_=cT[e * P : (e + 1) * P, 0:QC])
                    nc.scalar.dma_start(out=x_sb[e][:, 0:QC],
                                        in_=xT[e * P : (e + 1) * P, 0:QC])
                if act_warm:
                    # preload the Exp activation table while DMAs stream
                    warm = small.tile([1, 8], f32, tag="warm")
                    nc.vector.memset(warm[:], 0.0)
                    warm2 = small.tile([1, 8], bf16, tag="warm2")
                    nc.scalar.activation(
                        warm2[:], warm[:], mybir.ActivationFunctionType.Exp
                    )
                emit_ktproj(0)
                emit_qtproj(0)
                nc.gpsimd.memset(v_sb[:, :, :, D], 1.0)
                eb_first = eb_fetch(0, 0)
                dma_c(QC, 2 * QC)
                emit_ktproj(1)
                dma_c(2 * QC, M)
                eb_second = eb_fetch(1, 0)
                dma_x(QC, N)
                nc.sync.dma_start(out=wo_sb[:, 0, :], in_=woT[0:P, :])
                nc.sync.dma_start(out=wo_sb[0 : CG - P, 1, :], in_=woT[P:CG, :])
                fill_q.append(gen_ktproj(2))
                fill_q.append(gen_ktproj(3))
            else:
                nc.sync.dma_start(out=wk_sb[:], in_=wkT.rearrange("p (c d) -> p c d", d=CG))
                dma_c(0, M)
                nc.sync.dma_start(out=wv_sb[:], in_=wvT.rearrange("p (c d) -> p c d", d=CG))
                nc.sync.dma_start(out=wq_sb[:], in_=wqT.rearrange("p (c d) -> p c d", d=CG))
                nc.sync.dma_start(out=wo_sb[:, 0, :], in_=woT[0:P, :])
                nc.sync.dma_start(out=wo_sb[0 : CG - P, 1, :], in_=woT[P:CG, :])
                dma_x(0, N)
                nc.vector.memset(v_sb[:, :, :, D], 1.0)
                for jq_ in range(NQ):
                    emit_ktproj(jq_)
                emit_qtproj(0)

            for jq in range(NQ):
                qs = slice(jq * QC, (jq + 1) * QC)
                if pieces and jq > 0:
                    if jq + 1 < NQ:
                        fill_q.append(gen_qtproj(jq + 1))
                    # rebalance: later jqs have less projection filler, so
                    # defer part of the y-group work toward them
                    ysched = {1: [0, 1, 2, 3], 2: [4, 5],
                              3: [6, 7, 8, 9, 10, 11]}[jq]
                    for t in ysched:
                        fill_q.append(gen_ygroup(t))

                # ---- segment 0: heads 0/1, paired per m-tile as PE row
                # tiles (0,0)/(64,0) -> concurrent 64-row matmuls ----
                if jq == 0 and eb_first is not None:
                    eb_lo, eb_hi = eb_first, eb_second
                else:
                    eb_lo, eb_hi = eb_fetch(0, jq), None
                o_pair = ps_o.tile([D + 1, 2 * QC], f32, tag="ops", name="o_pair")

                def emit_pv01(mt_, pt_):
                    nc.tensor.matmul(
                        o_pair[:, 0:QC], v_sb[:, mt_, 0, :], pt_[:, 0:QC],
                        start=(mt_ == 0), stop=(mt_ == MT - 1),
                    )
                    nc.tensor.matmul(
                        o_pair[:, QC : 2 * QC], v_sb[:, mt_, 1, :],
                        pt_[:, QC : 2 * QC],
                        start=(mt_ == 0), stop=(mt_ == MT - 1),
                    )

                pending = []
                for mt in range(MT):
                    if pieces and jq == 0 and mt == 6 and NQ > 1:
                        fill_q.append(gen_qtproj(1))
                    if startup == 2 and jq == 0 and mt in (4, 8) and fill_q:
                        # K chunks 2/3 live in fill_q; finish each well
                        # before the S matmuls that read it
                        run_gen(fill_q.popleft())
                    if mt == 4 and eb_hi is None:
                        eb_hi = eb_fetch(1, jq)
                    sp = ps_sp.tile([P, 2 * QC], f32, tag="sp")
                    nc.tensor.matmul(
                        sp[:, 0:QC],
                        kt[0][0:D, mt * P : (mt + 1) * P],
                        qt[0][0:D, qs], start=True, stop=True,
                    )
                    nc.tensor.matmul(
                        sp[:, QC : 2 * QC],
                        kt[0][D:P, mt * P : (mt + 1) * P],
                        qt[0][D:P, qs], start=True, stop=True,
                    )
                    pt = pt_pool.tile([P, 2 * QC], bf16, tag="pt")
                    nc.scalar.activation(
                        pt[:], sp[:], mybir.ActivationFunctionType.Exp
                    )
                    ebs = eb_lo if mt < 8 else eb_hi
                    off = (mt % 8) * 2 * QC
                    nc.vector.tensor_mul(
                        pt[:], pt[:], ebs[:, off : off + 2 * QC]
                    )
                    if jq == 0:
                        emit_vproj(mt)
                    elif pieces:
                        drain(pieces)
                    pending.append((mt, pt))
                    if len(pending) > odelay and (
                            pvburst == 1 or mt % pvburst == pvburst - 1):
                        while len(pending) > odelay:
                            emit_pv01(*pending.pop(0))
                for po in pending:
                    emit_pv01(*po)
                # normalize heads 0/1
                for hh in range(2):
                    hsl = slice(hh * QC, (hh + 1) * QC)
                    recip = small.tile([1, QC], f32, tag="recip")
                    nc.vector.reciprocal(recip[:], o_pair[D : D + 1, hsl])
                    recip_bc = small.tile([D, QC], f32, tag="recipbc")
                    nc.gpsimd.partition_broadcast(recip_bc[:], recip[:])
                    nc.vector.tensor_mul(
                        ot01[hh * D : (hh + 1) * D, qs],
                        o_pair[0:D, hsl], recip_bc[:],
                    )

                # ---- segment 1: head 2, paired across m-tiles (replica in
                # the upper partition half drives tile (64,0)) ----
                eb_sb = eb_fetch(2, jq)
                o_ps = ps_o.tile([D + 1, QC], f32, tag="ops", name="o_ps2")

                def emit_opair(tp_, pt_):
                    for half_i in range(2):
                        mt_ = tp_ + half_i
                        nc.tensor.matmul(
                            o_ps[:], v_sb[:, mt_, 2, :],
                            pt_[:, half_i * QC : (half_i + 1) * QC],
                            start=(mt_ == 0), stop=(mt_ == MT - 1),
                        )

                pending = []
                for tp in range(0, MT, 2):
                    sp = ps_sp.tile([P, 2 * QC], f32, tag="sp")
                    nc.tensor.matmul(
                        sp[:, 0:QC],
                        kt[2][0:D, tp * P : (tp + 1) * P],
                        qt[2][0:D, qs], start=True, stop=True,
                    )
                    nc.tensor.matmul(
                        sp[:, QC : 2 * QC],
                        kt[2][D:P, (tp + 1) * P : (tp + 2) * P],
                        qt[2][D:P, qs], start=True, stop=True,
                    )
                    pt = pt_pool.tile([P, 2 * QC], bf16, tag="pt")
                    nc.scalar.activation(
                        pt[:], sp[:], mybir.ActivationFunctionType.Exp
                    )
                    nc.vector.tensor_mul(
                        pt[:], pt[:], eb_sb[:, tp * QC : (tp + 2) * QC]
                    )
                    if pieces:
                        drain(pieces)
                    pending.append((tp, pt))
                    if len(pending) > odelay:
                        emit_opair(*pending.pop(0))
                for po in pending:
                    emit_opair(*po)
                last_block = (jq == NQ - 1)
                if last_block and tail_split:
                    # finish per 128-q subtile so the final y groups
                    # pipeline with the remaining normalizes
                    for sub in range(QC // P):
                        ssl = slice(sub * P, (sub + 1) * P)
                        recip = small.tile([1, P], f32, tag="recip")
                        nc.vector.reciprocal(recip[:], o_ps[D : D + 1, ssl])
                        recip_bc = small.tile([D, P], f32, tag="recipbc")
                        nc.gpsimd.partition_broadcast(recip_bc[:], recip[:])
                        nc.vector.tensor_mul(
                            ot2[:, jq * QC + sub * P : jq * QC + (sub + 1) * P],
                            o_ps[0:D, ssl], recip_bc[:],
                        )
                        emit_ygroup((NQ - 1) * NQ + sub,
                                    act_copy=(sub % 2 == 0))
                else:
                    recip = small.tile([1, QC], f32, tag="recip")
                    nc.vector.reciprocal(recip[:], o_ps[D : D + 1, :])
                    recip_bc = small.tile([D, QC], f32, tag="recipbc")
                    nc.gpsimd.partition_broadcast(recip_bc[:], recip[:])
                    nc.vector.tensor_mul(ot2[:, qs], o_ps[0:D, :], recip_bc[:])
                drain_all()

            if not tail_split:
                for qq in range((NQ - 1) * NQ, NQ * NQ):
                    emit_ygroup(qq, act_copy=(qq % 2 == 0))

    nc.compile()
    return nc


def _shard_inputs(x, context, attn_bias, Wq, Wkv, Wout):
    scale = D ** -0.5
    in_maps = []
    for core in range(NCORES):
        b, g = divmod(core, GROUPS)
        cs = slice(g * CG, (g + 1) * CG)
        # exp(bias) quantized to u8 with a per-(h, n) scale: the softmax
        # denominator shares any per-query factor, so it cancels exactly.
        blog = attn_bias[b, g * HG : (g + 1) * HG]            # [HG, n, m]
        ebn = np.exp(blog - blog.max(axis=2, keepdims=True))  # (0, 1]
        u8v = np.rint(ebn * 255.0).astype(np.uint8).transpose(0, 2, 1)
        # head-2 slab: cols = mt*QC + qc
        h2 = (u8v[2].reshape(MT, P, NQ, QC).transpose(2, 1, 0, 3)
              .reshape(NQ, P, MT * QC))
        # head-0/1 pair slabs: cols = mt*2QC + h*QC + qc, split at m-tile 8
        pr = (u8v[:2].reshape(2, MT, P, NQ, QC).transpose(3, 2, 1, 0, 4)
              .reshape(NQ, P, MT * 2 * QC))
        ebT = np.stack([pr[..., : MT * QC], pr[..., MT * QC :], h2], axis=0)

        def wtile(w):
            # [E, CG] -> [P, EC*CG] so each partition line is one fat
            # contiguous DMA descriptor ("p (c d)" layout)
            return np.ascontiguousarray(
                w.reshape(EC, P, CG).transpose(1, 0, 2).reshape(P, EC * CG)
            ).astype(BF16)

        in_maps.append(
            {
                "xT": np.ascontiguousarray(x[b].T).astype(BF16),
                "cT": np.ascontiguousarray(context[b].T).astype(BF16),
                "expbT": np.ascontiguousarray(ebT),
                "wqT": wtile(Wq[cs, :].T * scale),
                "wkT": wtile(Wkv[cs, :].T),
                "wvT": wtile(Wkv[E + cs.start : E + cs.stop, :].T),
                "woT": np.ascontiguousarray(Wout[:, cs].T).astype(BF16),
            }
        )
    return in_maps


def kernel(x, context, attn_bias, Wq, Wkv, Wout, b_out):
    global _CACHED_NC
    if _CACHED_NC is None:
        _CACHED_NC = build_nc()
    nc = _CACHED_NC

    x = np.asarray(x, dtype=np.float32)
    context = np.asarray(context, dtype=np.float32)
    attn_bias = np.asarray(attn_bias, dtype=np.float32)
    Wq = np.asarray(Wq, dtype=np.float32)
    Wkv = np.asarray(Wkv, dtype=np.float32)
    Wout = np.asarray(Wout, dtype=np.float32)
    b_out = np.asarray(b_out, dtype=np.float32)

    in_maps = _shard_inputs(x, context, attn_bias, Wq, Wkv, Wout)
    try:
        res = run_bass_kernel_spmd(nc, in_maps, list(range(NCORES)))
    except Exception:
        # transient device failures have been observed on this fabric; give the
        # runtime one chance to reconnect before giving up
        import jax
        try:
            jax.clear_caches()
        except Exception:
            pass
        res = run_bass_kernel_spmd(nc, in_maps, list(range(NCORES)))

    out = np.zeros((B, N, E), dtype=np.float32)
    for core in range(NCORES):
        out[core // GROUPS] += np.asarray(res.results[core]["y"], dtype=np.float32)
    out += b_out.astype(np.float32)
    return out


# revision 8
# speedup vs baseline: 1.1397x; 1.1397x over previous
"""Trainium2 Bass kernel for 12-head cross-attention with additive bias.

Reference computation (b=2, n=m=2048, e=768, h=12, d=64), all fp32:
    q  = x @ Wq.T;  kv = context @ Wkv.T;  k, v = split(kv)
    sim  = (q_h @ k_h.T) * d**-0.5 + attn_bias
    out_h = softmax(sim) @ v_h
    y = concat_heads(out) @ Wout.T + b_out
Sharding: 8 cores = 2 batches x 4 head-groups (3 heads each).  Each core
computes the projections for its head group, attention for its 3 heads, and
a partial output projection y_part[b] = out_g @ Wout[:, cols_g].T.  The host
sums the 4 per-group partials of each batch and adds b_out.

Key device-side structure (all bf16 on-chip, softmax scale folded into Wq):
    QT[d,q] / KT[d,m] = WT-chunks.T @ xT-chunks          (PE)
    ST[m,q] = KT-slice.T @ QT  -- d=64 contraction, so every S matmul is a
        64-row PE tile.  Heads 0/1 are packed in the two partition halves of
        qt[0]/kt[0]; their S matmuls for the same m-tile are issued
        back-to-back as tile (0,0) / (64,0), which the PE runs CONCURRENTLY
        (2x row tiling).  Head 2 is replicated into both partition halves so
        two m-tiles of it pair the same way.
    PT = exp(ST)                                          (ScalarE, PSUM->SBUF)
    PT *= eb tile                                         (DVE, bf16 2x)
        exp(attn_bias) ships as uint8 with a per-(h, n) scale -- softmax
        normalization cancels any per-query factor exactly -- and the SWDGE
        DMA casts u8 -> bf16 on the way into SBUF (halves HBM reads).
    OT[d,q] += V_aug-slice.T @ PT   (V_aug has a ones column -> denominators)
    y[q,j] += OT-chunks.T @ WoutT-chunks   (normalized by 1/denom first)
"""

import numpy as np
import ml_dtypes

import concourse.bacc as bacc
import concourse.mybir as mybir
import concourse.tile as tile
from concourse.bass_utils import run_bass_kernel_spmd

BF16 = ml_dtypes.bfloat16

B, N, M, E = 2, 2048, 2048, 768
HEADS = 12
D = 64                      # head dim
GROUPS = 4                  # head groups (cores per batch)
HG = HEADS // GROUPS        # heads per group = 3
CG = HG * D                 # channels per group = 192
NCORES = 8

P = 128                     # partitions
QC = 512                    # q free-dim chunk
NQ = N // QC                # 4 q-chunks
MT = M // P                 # 16 m-tiles
EC = E // P                 # 6 contraction chunks
JC = 384                    # output-proj free chunk
NJ = E // JC                # 2 output-proj chunks

_CACHED_NC = None


def build_nc(reps=1, sp_bufs=2, gen_bufs=2, pt_bufs=8, eb_bufs=3,
             ydt_bf16=1, startup=2, eb_chunks=2, odelay=3, pieces=2,
             tail_split=1, act_warm=1, pvburst=1):
    import collections as _collections
    f32 = mybir.dt.float32
    bf16 = mybir.dt.bfloat16
    u8 = mybir.dt.uint8
    y_dt = bf16 if ydt_bf16 else f32

    nc = bacc.Bacc("TRN2", debug=False)
    xT = nc.dram_tensor("xT", [E, N], bf16, kind="ExternalInput")
    cT = nc.dram_tensor("cT", [E, M], bf16, kind="ExternalInput")
    # exp(bias) quantized to u8 with a per-(h, n) scale (softmax cancels any
    # per-query-column factor).  Slab 0: heads 0/1 interleaved per m-tile
    # (cols = mt*2QC + h*QC + qc) for m-tiles 0-7; slab 1: same, m-tiles
    # 8-15; slab 2: head 2 (cols = mt*QC + qc).
    expbT = nc.dram_tensor("expbT", [HG, NQ, P, MT * QC], u8,
                           kind="ExternalInput")
    # w{q,k,v}T pre-tiled on host to [P, EC*CG]: one fat descriptor per
    # partition line instead of EC*CG/P thin ones
    wqT = nc.dram_tensor("wqT", [P, EC * CG], bf16, kind="ExternalInput")
    wkT = nc.dram_tensor("wkT", [P, EC * CG], bf16, kind="ExternalInput")
    wvT = nc.dram_tensor("wvT", [P, EC * CG], bf16, kind="ExternalInput")
    woT = nc.dram_tensor("woT", [CG, E], bf16, kind="ExternalInput")
    y = nc.dram_tensor("y", [N, E], y_dt, kind="ExternalOutput")

    with tile.TileContext(nc) as tc:
        with (
            tc.tile_pool(name="big", bufs=1) as big,
            tc.tile_pool(name="eb_pool", bufs=eb_bufs) as eb_pool,
            tc.tile_pool(name="pt_pool", bufs=pt_bufs) as pt_pool,
            tc.tile_pool(name="small", bufs=4) as small,
            tc.tile_pool(name="ysb_pool", bufs=4) as ysb_pool,
            tc.tile_pool(name="ps_sp", bufs=sp_bufs, space="PSUM") as ps_sp,
            tc.tile_pool(name="ps_gen", bufs=gen_bufs, space="PSUM") as ps_gen,
            # bufs=1: o_pair is a 2-bank tile; PSUM budget is
            # sp 2x2 + gen 2x1 + o 1x2 = 8 banks
            tc.tile_pool(name="ps_o", bufs=1, space="PSUM") as ps_o,
        ):
          for _rep in range(reps):
            # ---- load inputs (context first: KT/V unblock the S matmuls) ----
            wq_sb = big.tile([P, EC, CG], bf16)
            wk_sb = big.tile([P, EC, CG], bf16)
            wv_sb = big.tile([P, EC, CG], bf16)
            wo_sb = big.tile([P, 2, E], bf16)
            c_sb = [big.tile([P, M], bf16, name=f"c{_e}") for _e in range(EC)]
            x_sb = [big.tile([P, N], bf16, name=f"x{_e}") for _e in range(EC)]

            def dma_c(q0, q1):
                for e in range(EC):
                    nc.sync.dma_start(out=c_sb[e][:, q0:q1],
                                      in_=cT[e * P : (e + 1) * P, q0:q1])

            def dma_x(q0, q1):
                for e in range(EC):
                    nc.sync.dma_start(out=x_sb[e][:, q0:q1],
                                      in_=xT[e * P : (e + 1) * P, q0:q1])

            qt = [big.tile([P, N], bf16, name=f"qt{_h}") for _h in (0, 2)]
            qt = {0: qt[0], 2: qt[1]}
            ot01 = big.tile([P, N], bf16)
            ot2 = big.tile([D, N], bf16)

            kt = [big.tile([P, M], bf16, name=f"kt{_h}") for _h in (0, 2)]
            kt = {0: kt[0], 2: kt[1]}

            def gen_proj(jq_, w_sb, dst, src):
                # Q or K projection for one 512-chunk, as resumable pieces.
                # Heads 0/1 stay packed in the two partition halves of
                # dst[0]; head 2 lands in dst[2][0:D] and is replicated to
                # dst[2][D:P] so its S matmuls can pair as PE row tiles.
                qs_ = slice(jq_ * QC, (jq_ + 1) * QC)
                pr01 = ps_gen.tile([P, QC], f32, tag="gen", name="pr01")
                for e in range(EC):
                    nc.tensor.matmul(
                        pr01[:], w_sb[:, e, 0:P], src[e][:, qs_],
                        start=(e == 0), stop=(e == EC - 1),
                    )
                    yield
                nc.vector.tensor_copy(dst[0][:, qs_], pr01[:])
                yield
                pr2 = ps_gen.tile([D, QC], f32, tag="gen", name="pr2")
                for e in range(EC):
                    nc.tensor.matmul(
                        pr2[:], w_sb[:, e, P:CG], src[e][:, qs_],
                        start=(e == 0), stop=(e == EC - 1),
                    )
                    yield
                nc.vector.tensor_copy(dst[2][0:D, qs_], pr2[:])
                yield
                nc.vector.tensor_copy(dst[2][D:P, qs_], dst[2][0:D, qs_])
                yield

            def gen_qtproj(jq_):
                return gen_proj(jq_, wq_sb, qt, x_sb)

            def gen_ktproj(jq_):
                return gen_proj(jq_, wk_sb, kt, c_sb)

            def run_gen(g):
                for _ in g:
                    pass

            def emit_qtproj(jq_):
                run_gen(gen_qtproj(jq_))

            def emit_ktproj(jq_):
                run_gen(gen_ktproj(jq_))

            fill_q = _collections.deque()

            def drain(n):
                for _ in range(n):
                    while fill_q:
                        try:
                            next(fill_q[0])
                            break
                        except StopIteration:
                            fill_q.popleft()
                    if not fill_q:
                        return

            def drain_all():
                while fill_q:
                    run_gen(fill_q.popleft())

            def eb_fetch(h_, jq_):
                # u8 -> bf16 cast during DMA: SWDGE (gpsimd) only
                eb_sb = eb_pool.tile([P, MT * QC], bf16, tag="eb")
                step = MT * QC // eb_chunks
                for ci in range(eb_chunks):
                    nc.gpsimd.dma_start(
                        out=eb_sb[:, ci * step : (ci + 1) * step],
                        in_=expbT[h_, jq_, :, ci * step : (ci + 1) * step],
                    )
                return eb_sb

            # ---- V projection helper (emitted per m-tile inside jq0) ----
            v_sb = big.tile([P, MT, HG, D + 1], bf16)

            def emit_vproj(mt):
                pv = ps_gen.tile([P, CG], f32, tag="gen", name="pv")
                for e in range(EC):
                    nc.tensor.matmul(
                        pv[:], c_sb[e][:, mt * P : (mt + 1) * P], wv_sb[:, e, :],
                        start=(e == 0), stop=(e == EC - 1),
                    )
                nc.vector.tensor_copy(
                    v_sb[:, mt, :, 0:D], pv.rearrange("p (h d) -> p h d", d=D)
                )

            def gen_ygroup(qtile, act_copy=False):
                qsl = slice(qtile * P, (qtile + 1) * P)
                y_sb = ysb_pool.tile([P, E], y_dt, tag="ysb", name="y_sb")
                for jn in range(NJ):
                    jsl = slice(jn * JC, (jn + 1) * JC)
                    y_ps = ps_gen.tile([P, JC], f32, tag="gen", name="y_ps")
                    nc.tensor.matmul(
                        y_ps[:], ot01[:, qsl], wo_sb[:, 0, jsl],
                        start=True, stop=False,
                    )
                    yield
                    nc.tensor.matmul(
                        y_ps[:], ot2[:, qsl], wo_sb[0 : CG - P, 1, jsl],
                        start=False, stop=True,
                    )
                    yield
                    if act_copy:
                        nc.scalar.copy(y_sb[:, jsl], y_ps[:])
                    else:
                        nc.vector.tensor_copy(y_sb[:, jsl], y_ps[:])
                    yield
                nc.sync.dma_start(out=y[qsl, :], in_=y_sb[:])

            def emit_ygroup(qtile, act_copy=False):
                run_gen(gen_ygroup(qtile, act_copy))

            eb_first = None
            eb_second = None
            if startup == 2:
                # startup-critical loads, spread across three sequencers so
                # DMA issue (~0.6us each) pipelines: K path on SP, Q path on
                # ACT, V on DVE-side SWDGE
                nc.sync.dma_start(out=wk_sb[:], in_=wkT.rearrange("p (c d) -> p c d", d=CG))
                nc.scalar.dma_start(out=wq_sb[:], in_=wqT.rearrange("p (c d) -> p c d", d=CG))
                nc.gpsimd.dma_start(out=wv_sb[:], in_=wvT.rearrange("p (c d) -> p c d", d=CG))
                for e in range(EC):
                    nc.sync.dma_start(out=c_sb[e][:, 0:QC],
                                      in_=cT[e * P : (e + 1) * P, 0:QC])
                    nc.scalar.dma_start(out=x_sb[e][:, 0:QC],
                                        in_=xT[e * P : (e + 1) * P, 0:QC])
                if act_warm:
                    # preload the Exp activation table while DMAs stream
                    warm = small.tile([1, 8], f32, tag="warm")
                    nc.vector.memset(warm[:], 0.0)
                    warm2 = small.tile([1, 8], bf16, tag="warm2")
                    nc.scalar.activation(
                        warm2[:], warm[:], mybir.ActivationFunctionType.Exp
                    )
                emit_ktproj(0)
                emit_qtproj(0)
                nc.gpsimd.memset(v_sb[:, :, :, D], 1.0)
                eb_first = eb_fetch(0, 0)
                dma_c(QC, 2 * QC)
                emit_ktproj(1)
                dma_c(2 * QC, M)
                eb_second = eb_fetch(1, 0)
                dma_x(QC, N)
                nc.sync.dma_start(out=wo_sb[:, 0, :], in_=woT[0:P, :])
                nc.sync.dma_start(out=wo_sb[0 : CG - P, 1, :], in_=woT[P:CG, :])
                fill_q.append(gen_ktproj(2))
                fill_q.append(gen_ktproj(3))
            else:
                nc.sync.dma_start(out=wk_sb[:], in_=wkT.rearrange("p (c d) -> p c d", d=CG))
                dma_c(0, M)
                nc.sync.dma_start(out=wv_sb[:], in_=wvT.rearrange("p (c d) -> p c d", d=CG))
                nc.sync.dma_start(out=wq_sb[:], in_=wqT.rearrange("p (c d) -> p c d", d=CG))
                nc.sync.dma_start(out=wo_sb[:, 0, :], in_=woT[0:P, :])
                nc.sync.dma_start(out=wo_sb[0 : CG - P, 1, :], in_=woT[P:CG, :])
                dma_x(0, N)
                nc.vector.memset(v_sb[:, :, :, D], 1.0)
                for jq_ in range(NQ):
                    emit_ktproj(jq_)
                emit_qtproj(0)

            for jq in range(NQ):
                qs = slice(jq * QC, (jq + 1) * QC)
                if pieces and jq > 0:
                    if jq + 1 < NQ:
                        fill_q.append(gen_qtproj(jq + 1))
                    # rebalance: later jqs have less projection filler, so
                    # defer part of the y-group work toward them
                    ysched = {1: [0, 1, 2, 3], 2: [4, 5],
                              3: [6, 7, 8, 9, 10, 11]}[jq]
                    for t in ysched:
                        fill_q.append(gen_ygroup(t))

                # ---- segment 0: heads 0/1, paired per m-tile as PE row
                # tiles (0,0)/(64,0) -> concurrent 64-row matmuls ----
                if jq == 0 and eb_first is not None:
                    eb_lo, eb_hi = eb_first, eb_second
                else:
                    eb_lo, eb_hi = eb_fetch(0, jq), None
                o_pair = ps_o.tile([D + 1, 2 * QC], f32, tag="ops", name="o_pair")

                def emit_pv01(mt_, pt_):
                    nc.tensor.matmul(
                        o_pair[:, 0:QC], v_sb[:, mt_, 0, :], pt_[:, 0:QC],
                        start=(mt_ == 0), stop=(mt_ == MT - 1),
                    )
                    nc.tensor.matmul(
                        o_pair[:, QC : 2 * QC], v_sb[:, mt_, 1, :],
                        pt_[:, QC : 2 * QC],
                        start=(mt_ == 0), stop=(mt_ == MT - 1),
                    )

                pending = []
                for mt in range(MT):
                    if pieces and jq == 0 and mt == 6 and NQ > 1:
                        fill_q.append(gen_qtproj(1))
                    if startup == 2 and jq == 0 and mt in (4, 8) and fill_q:
                        # K chunks 2/3 live in fill_q; finish each well
                        # before the S matmuls that read it
                        run_gen(fill_q.popleft())
                    if mt == 4 and eb_hi is None:
                        eb_hi = eb_fetch(1, jq)
                    sp = ps_sp.tile([P, 2 * QC], f32, tag="sp")
                    nc.tensor.matmul(
                        sp[:, 0:QC],
                        kt[0][0:D, mt * P : (mt + 1) * P],
                        qt[0][0:D, qs], start=True, stop=True,
                    )
                    nc.tensor.matmul(
                        sp[:, QC : 2 * QC],
                        kt[0][D:P, mt * P : (mt + 1) * P],
                        qt[0][D:P, qs], start=True, stop=True,
                    )
                    pt = pt_pool.tile([P, 2 * QC], bf16, tag="pt")
                    nc.scalar.activation(
                        pt[:], sp[:], mybir.ActivationFunctionType.Exp
                    )
                    ebs = eb_lo if mt < 8 else eb_hi
                    off = (mt % 8) * 2 * QC
                    nc.vector.tensor_mul(
                        pt[:], pt[:], ebs[:, off : off + 2 * QC]
                    )
                    if jq == 0:
                        emit_vproj(mt)
                    elif pieces:
                        drain(pieces)
                    pending.append((mt, pt))
                    if len(pending) > odelay and (
                            pvburst == 1 or mt % pvburst == pvburst - 1):
                        while len(pending) > odelay:
                            emit_pv01(*pending.pop(0))
                for po in pending:
                    emit_pv01(*po)
                # normalize heads 0/1
                for hh in range(2):
                    hsl = slice(hh * QC, (hh + 1) * QC)
                    recip = small.tile([1, QC], f32, tag="recip")
                    nc.vector.reciprocal(recip[:], o_pair[D : D + 1, hsl])
                    recip_bc = small.tile([D, QC], f32, tag="recipbc")
                    nc.gpsimd.partition_broadcast(recip_bc[:], recip[:])
                    nc.vector.tensor_mul(
                        ot01[hh * D : (hh + 1) * D, qs],
                        o_pair[0:D, hsl], recip_bc[:],
                    )

                # ---- segment 1: head 2, paired across m-tiles (replica in
                # the upper partition half drives tile (64,0)) ----
                eb_sb = eb_fetch(2, jq)
                o_ps = ps_o.tile([D + 1, QC], f32, tag="ops", name="o_ps2")

                def emit_opair(tp_, pt_):
                    for half_i in range(2):
                        mt_ = tp_ + half_i
                        nc.tensor.matmul(
                            o_ps[:], v_sb[:, mt_, 2, :],
                            pt_[:, half_i * QC : (half_i + 1) * QC],
                            start=(mt_ == 0), stop=(mt_ == MT - 1),
                        )

                pending = []
                for tp in range(0, MT, 2):
                    sp = ps_sp.tile([P, 2 * QC], f32, tag="sp")
                    nc.tensor.matmul(
                        sp[:, 0:QC],
                        kt[2][0:D, tp * P : (tp + 1) * P],
                        qt[2][0:D, qs], start=True, stop=True,
                    )
                    nc.tensor.matmul(
                        sp[:, QC : 2 * QC],
                        kt[2][D:P, (tp + 1) * P : (tp + 2) * P],
                        qt[2][D:P, qs], start=True, stop=True,
                    )
                    pt = pt_pool.tile([P, 2 * QC], bf16, tag="pt")
                    nc.scalar.activation(
                        pt[:], sp[:], mybir.ActivationFunctionType.Exp
                    )
                    nc.vector.tensor_mul(
                        pt[:], pt[:], eb_sb[:, tp * QC : (tp + 2) * QC]
                    )
                    if pieces:
                        drain(pieces)
                    pending.append((tp, pt))
                    if len(pending) > odelay:
                        emit_opair(*pending.pop(0))
                for po in pending:
                    emit_opair(*po)
                last_block = (jq == NQ - 1)
                if last_block and tail_split:
                    # finish per 128-q subtile so the final y groups
                    # pipeline with the remaining normalizes
                    for sub in range(QC // P):
                        ssl = slice(sub * P, (sub + 1) * P)
                        recip = small.tile([1, P], f32, tag="recip")
                        nc.vector.reciprocal(recip[:], o_ps[D : D + 1, ssl])
                        recip_bc = small.tile([D, P], f32, tag="recipbc")
                        nc.gpsimd.partition_broadcast(recip_bc[:], recip[:])
                        nc.vector.tensor_mul(
                            ot2[:, jq * QC + sub * P : jq * QC + (sub + 1) * P],
                            o_ps[0:D, ssl], recip_bc[:],
                        )
                        emit_ygroup((NQ - 1) * NQ + sub,
                                    act_copy=(sub % 2 == 0))
                else:
                    recip = small.tile([1, QC], f32, tag="recip")
                    nc.vector.reciprocal(recip[:], o_ps[D : D + 1, :])
                    recip_bc = small.tile([D, QC], f32, tag="recipbc")
                    nc.gpsimd.partition_broadcast(recip_bc[:], recip[:])
                    nc.vector.tensor_mul(ot2[:, qs], o_ps[0:D, :], recip_bc[:])
                drain_all()

            if not tail_split:
                for qq in range((NQ - 1) * NQ, NQ * NQ):
                    emit_ygroup(qq, act_copy=(qq % 2 == 0))

    nc.compile()
    return nc


def _shard_inputs(x, context, attn_bias, Wq, Wkv, Wout):
    scale = D ** -0.5
    in_maps = []
    for core in range(NCORES):
        b, g = divmod(core, GROUPS)
        cs = slice(g * CG, (g + 1) * CG)
        # exp(bias) quantized to u8 with a per-(h, n) scale: the softmax
        # denominator shares any per-query factor, so it cancels exactly.
        blog = attn_bias[b, g * HG : (g + 1) * HG]            # [HG, n, m]
        ebn = np.exp(blog - blog.max(axis=2, keepdims=True))  # (0, 1]
        u8v = np.rint(ebn * 255.0).astype(np.uint8).transpose(0, 2, 1)
        # head-2 slab: cols = mt*QC + qc
        h2 = (u8v[2].reshape(MT, P, NQ, QC).transpose(2, 1, 0, 3)
              .reshape(NQ, P, MT * QC))
        # head-0/1 pair slabs: cols = mt*2QC + h*QC + qc, split at m-tile 8
        pr = (u8v[:2].reshape(2, MT, P, NQ, QC).transpose(3, 2, 1, 0, 4)
              .reshape(NQ, P, MT * 2 * QC))
        ebT = np.stack([pr[..., : MT * QC], pr[..., MT * QC :], h2], axis=0)

        def wtile(w):
            # [E, CG] -> [P, EC*CG] so each partition line is one fat
            # contiguous DMA descriptor ("p (c d)" layout)
            return np.ascontiguousarray(
                w.reshape(EC, P, CG).transpose(1, 0, 2).reshape(P, EC * CG)
            ).astype(BF16)

        in_maps.append(
            {
                "xT": np.ascontiguousarray(x[b].T).astype(BF16),
                "cT": np.ascontiguousarray(context[b].T).astype(BF16),
                "expbT": np.ascontiguousarray(ebT),
                "wqT": wtile(Wq[cs, :].T * scale),
                "wkT": wtile(Wkv[cs, :].T),
                "wvT": wtile(Wkv[E + cs.start : E + cs.stop, :].T),
                "woT": np.ascontiguousarray(Wout[:, cs].T).astype(BF16),
            }
        )
    return in_maps


def kernel(x, context, attn_bias, Wq, Wkv, Wout, b_out):
    global _CACHED_NC
    if _CACHED_NC is None:
        _CACHED_NC = build_nc()
    nc = _CACHED_NC

    x = np.asarray(x, dtype=np.float32)
    context = np.asarray(context, dtype=np.float32)
    attn_bias = np.asarray(attn_bias, dtype=np.float32)
    Wq = np.asarray(Wq, dtype=np.float32)
    Wkv = np.asarray(Wkv, dtype=np.float32)
    Wout = np.asarray(Wout, dtype=np.float32)
    b_out = np.asarray(b_out, dtype=np.float32)

    in_maps = _shard_inputs(x, context, attn_bias, Wq, Wkv, Wout)
    try:
        res = run_bass_kernel_spmd(nc, in_maps, list(range(NCORES)))
    except Exception:
        # transient device failures have been observed on this fabric; give the
        # runtime one chance to reconnect before giving up
        import jax
        try:
            jax.clear_caches()
        except Exception:
            pass
        res = run_bass_kernel_spmd(nc, in_maps, list(range(NCORES)))

    out = np.zeros((B, N, E), dtype=np.float32)
    for core in range(NCORES):
        out[core // GROUPS] += np.asarray(res.results[core]["y"], dtype=np.float32)
    out += b_out.astype(np.float32)
    return out
